# revision 37
# baseline (speedup 1.0000x reference)
"""Trainium2 Bass kernel for nn_MultiAtrAttention.

Math (reference):
  sf[b,a]  = atr[b,a] * X[b]
  score    = (sf @ sf^T)/sqrt(E), masked_fill(outer(mask)==0, 0)
  attn     = softmax(score - global_max, axis=-1)            -> output 2
  attn_out = attn @ sf                                        (B*A,T,E)
  output   = attn_out.reshape(B,T,A*E) @ W.T + bias           -> output 1

Identities used:
  score[b,a] = c1*Gm[b],  c1 = atr^2/sqrt(E),  Gm = Xm@Xm^T,
               Xm = diag(mask)@X  (exact masking via zeroed rows/cols)
  softmax is shift-invariant -> drop the global-max (scores <= ~25, exp
    stays finite in f32);  P = exp(c1*Gm) is SYMMETRIC
  D = rowsum(P) comes free from the ACT accumulator
  ao = attn_out[b,a] = diag(fac) @ P @ X[b],  fac = atr/D
  The reference reshape (B*A,T,E)->(B,T,A*E) regroups rows:
    output[b, 64a+r, :] = ao.reshape(64, A*E)[r] @ W.T + bias
  i.e. per (b,a): out rows 64a..64a+63 = ao[8r+j, e] contracted with
    W[:, j*E+e].  lhsT chunks are stride-8 column slices of ao^T, where
    ao^T = X^T @ P @ diag(fac)  (uses P symmetric; no transposes needed)

Precision: the Gram/softmax path is full f32 (PE fp32 matmul = 4
cycles/row).  The value path (U^T = X^T@P and the output projection) runs
in float32r (1 cycle/row, ~1.2e-4 input rounding), giving ~2.5e-4 max rel
error on both outputs -- the PE would otherwise be 2.5x over the ~62us/core
HBM roofline that bounds this kernel (attn output alone is 16.8 MB/core).

Sharding: batch-parallel, 2 batches per core x 8 cores.
"""

import sys

sys.path.insert(0, "/opt/trn_rl_repo")

import numpy as np

import concourse.mybir as mybir
import concourse.tile as tile
from concourse import bacc
from concourse.bass import ds, ts
from concourse.bass_utils import run_bass_kernel_spmd

B, T, E, A = 16, 512, 256, 8
NB = B // 8          # batches per core
TC = T // 128        # t-chunks of 128
EC = E // 128        # e-chunks of 128
NP = NB * A          # (b, a) pairs per core
RB = T // A          # 64 output rows per (b, a) block
F32 = mybir.dt.float32

LAST_RESULTS = None  # test.py reads exec_time/trace from here


def _build_program(ppb=8, apb=4, vpb=2, spb=4, fpb=4, opb=2, psgb=2, psub=3, pspb=2):
    nc = bacc.Bacc()

    xn = nc.dram_tensor("xn", (NB, T, E), mybir.dt.float32r, kind="ExternalInput")
    xmt = nc.dram_tensor("xmt", (NB, E, T), F32, kind="ExternalInput")
    wt = nc.dram_tensor("wt", (A * E, E), mybir.dt.float32r, kind="ExternalInput")
    bias = nc.dram_tensor("bias", (1, E), F32, kind="ExternalInput")
    coef = nc.dram_tensor("coef", (1, 2 * NP), F32, kind="ExternalInput")
    attn_o = nc.dram_tensor("attn_o", (NP, T, T), F32, kind="ExternalOutput")
    out_o = nc.dram_tensor("out_o", (NB, T, E), F32, kind="ExternalOutput")

    F32R = mybir.dt.float32r
    Exp = mybir.ActivationFunctionType.Exp
    Mult = mybir.AluOpType.mult

    with tile.TileContext(nc) as tc:
        with tc.tile_pool(name="const", bufs=1) as const:
            ones = const.tile([1, 128], F32)
            nc.vector.memset(ones[:], 1.0)
            coef_row = const.tile([1, 2 * NP], F32)
            nc.sync.dma_start(coef_row[:], coef[:])
            bias_row = const.tile([1, E], F32)
            nc.sync.dma_start(bias_row[:], bias[:])
            coefb = const.tile([128, 2 * NP], F32)
            biasb = const.tile([128, E], F32)

            # broadcast per-pair scalars + bias to all 128 partitions
            with tc.tile_pool(name="setup_ps", bufs=1, space="PSUM") as sps:
                t1 = sps.tile([128, 2 * NP], F32, tag="c")
                nc.tensor.matmul(t1[:], ones[:], coef_row[:])
                nc.vector.tensor_copy(coefb[:], t1[:])
                t2 = sps.tile([128, E], F32, tag="b")
                nc.tensor.matmul(t2[:], ones[:], bias_row[:])
                nc.vector.tensor_copy(biasb[:], t2[:])

            # persistent operands; xmt gates the Gram matmuls -> load first
            xmt_sb = const.tile([128, NB, EC, T], F32)
            xn_sb = const.tile([128, NB, TC, E], F32R)
            wt_sb = const.tile([128, A * EC, E], F32R)
            for b in range(NB):
                nc.sync.dma_start(
                    xmt_sb[:, b], xmt[b].rearrange("(ec p) t -> p ec t", p=128)
                )
            for b in range(NB):
                nc.sync.dma_start(
                    xn_sb[:, b], xn[b].rearrange("(tc p) e -> p tc e", p=128)
                )
            nc.sync.dma_start(wt_sb[:], wt.rearrange("(k p) o -> p k o", p=128))
            g_sb = const.tile([128, NB, TC, T], F32)

            with (
                tc.tile_pool(name="ppool", bufs=ppb) as ppool,
                tc.tile_pool(name="apool", bufs=apb) as apool,
                tc.tile_pool(name="vpool", bufs=vpb) as vpool,
                tc.tile_pool(name="spool", bufs=spb) as spool,
                tc.tile_pool(name="fpool", bufs=fpb) as fpool,
                tc.tile_pool(name="opool", bufs=opb) as opool,
                tc.tile_pool(name="psg", bufs=psgb, space="PSUM") as psg,
                tc.tile_pool(name="psu", bufs=psub, space="PSUM") as psu,
                tc.tile_pool(name="psp", bufs=pspb, space="PSUM") as psp,
            ):
                # ---- Gram matrices Gm[b] = Xm @ Xm^T (shared by all attrs)
                for b in range(NB):
                    for ti in range(TC):
                        pg = psg.tile([128, T], F32, tag="pg")
                        for ec in range(EC):
                            nc.tensor.matmul(
                                pg[:],
                                xmt_sb[:, b, ec, ts(ti, 128)],
                                xmt_sb[:, b, ec, :],
                                start=(ec == 0),
                                stop=(ec == EC - 1),
                            )
                        nc.vector.tensor_copy(g_sb[:, b, ti, :], pg[:])

                def emit_proj(v2, a):
                    # out[b, 64a+r, o] = sum_{j,e} ao_b[8r+j, e]*W[o, j*E+e]
                    po = psp.tile([128, E], F32, tag="po")
                    for kc in range(A * EC):
                        jj, ec = kc // EC, kc % EC
                        nc.tensor.matmul(
                            po[:],
                            v2[:, ec, :, :].rearrange("p b t -> p (b t)")[
                                :, jj : jj + 8 * (NB * RB - 1) + 1 : 8
                            ],
                            wt_sb[:, kc, :],
                            start=(kc == 0),
                            stop=(kc == A * EC - 1),
                        )
                    out_sb = opool.tile([128, E], F32, tag="out")
                    nc.vector.tensor_tensor(
                        out_sb[:], po[:], biasb[:], op=mybir.AluOpType.add
                    )
                    for bb in range(NB):
                        nc.sync.dma_start(
                            out_o[bb, ds(RB * a, RB), :],
                            out_sb[bb * RB : (bb + 1) * RB, :],
                        )

                prev_v2, prev_a = None, None
                for a in range(A):
                    v2 = vpool.tile([128, EC, NB, T], F32R, tag="v2")
                    for b in range(NB):
                        j = b * A + a
                        # ---- P = exp(c1*Gm); D = rowsum(P) fused on ACT
                        p_sb = ppool.tile([128, TC, T], F32R, tag="p")
                        d_sb = spool.tile([128, TC], F32, tag="d")
                        for ti in range(TC):
                            nc.scalar.activation(
                                p_sb[:, ti, :],
                                g_sb[:, b, ti, :],
                                Exp,
                                scale=coefb[:, j : j + 1],
                                accum_out=d_sb[:, ti : ti + 1],
                            )
                        rd_sb = spool.tile([128, TC], F32, tag="rd")
                        fac_sb = spool.tile([128, TC], F32, tag="fac")
                        facr = fpool.tile([1, T], F32, tag="facr")
                        fr = fpool.tile([128, T], F32, tag="fr")
                        # latency-critical chain: gates v' -> proj; boost its
                        # priority so it jumps same-engine throughput work
                        with tc.high_priority(offset=60):
                            nc.vector.reciprocal(rd_sb[:], d_sb[:])
                            nc.vector.tensor_scalar_mul(
                                fac_sb[:], rd_sb[:], coefb[:, NP + j : NP + j + 1]
                            )
                            # (128,TC) column -> (1,512) row in p-major order
                            nc.sync.dma_start(facr[:], fac_sb[:])
                            nc.gpsimd.partition_broadcast(fr[:], facr[:])
                        # natural-t view of the p-major row: t = c*128 + p
                        fr_nat = fr[:, :].rearrange("e (p c) -> e c p", p=128, c=TC)

                        # ---- ao^T = X^T @ P @ diag(fac)   (P symmetric)
                        for ec in range(EC):
                            pu = psu.tile([128, T], F32, tag="pu")
                            for k in range(TC):
                                nc.tensor.matmul(
                                    pu[:],
                                    xn_sb[:, b, k, ts(ec, 128)],
                                    p_sb[:, k, :],
                                    start=(k == 0),
                                    stop=(k == TC - 1),
                                )
                            with tc.high_priority(offset=60):
                                nc.vector.tensor_tensor(
                                    v2[:, ec, b, :].rearrange(
                                        "e (c p) -> e c p", c=TC, p=128
                                    ),
                                    pu[:, :].rearrange(
                                        "e (c p) -> e c p", c=TC, p=128
                                    ),
                                    fr_nat,
                                    op=Mult,
                                )

                        # ---- attn = P/D -> HBM (row scale; emitted after the
                        # PE-feeding chain so the scheduler favors the latter)
                        att_sb = apool.tile([128, TC, T], F32, tag="att")
                        for ti in range(TC):
                            eng = nc.vector if ti < 2 else nc.gpsimd
                            eng.tensor_scalar_mul(
                                att_sb[:, ti, :],
                                p_sb[:, ti, :].bitcast(F32),
                                rd_sb[:, ti : ti + 1],
                            )
                        for h in range(2):
                            nc.sync.dma_start(
                                attn_o[j, ds(h * 256, 256)].rearrange(
                                    "(tc p) s -> p tc s", p=128
                                ),
                                att_sb[:, ds(h * 2, 2), :],
                            )

                    if prev_v2 is not None:
                        emit_proj(prev_v2, prev_a)
                    prev_v2, prev_a = v2, a
                emit_proj(prev_v2, prev_a)

    nc.finalize()
    return nc


_PROGRAM = None


def kernel(sent_feat, mask, atr_scores, W, b):
    global _PROGRAM, LAST_RESULTS
    sent_feat = np.asarray(sent_feat, dtype=np.float32)
    mask = np.asarray(mask)
    atr_scores = np.asarray(atr_scores, dtype=np.float32)
    W = np.asarray(W, dtype=np.float32)
    b = np.asarray(b, dtype=np.float32)

    xm = sent_feat * mask.astype(np.float32)[:, :, None]          # (B,T,E)
    xmt = np.ascontiguousarray(xm.transpose(0, 2, 1))             # (B,E,T)
    wt = np.ascontiguousarray(W.T)                                # (A*E, E)
    bias_row = np.ascontiguousarray(b.reshape(1, E))
    c1 = (atr_scores**2 / np.sqrt(E)).astype(np.float32)          # (B,A)
    c2 = atr_scores.astype(np.float32)

    if _PROGRAM is None:
        _PROGRAM = _build_program()
    nc = _PROGRAM

    in_maps = []
    for c in range(8):
        bs = slice(c * NB, (c + 1) * NB)
        coef = np.concatenate(
            [c1[bs].reshape(-1), c2[bs].reshape(-1)]
        ).reshape(1, 2 * NP)
        in_maps.append(
            {
                "xn": np.ascontiguousarray(sent_feat[bs]),
                "xmt": np.ascontiguousarray(xmt[bs]),
                "wt": wt,
                "bias": bias_row,
                "coef": np.ascontiguousarray(coef),
            }
        )

    last_exc = None
    for _attempt in range(3):
        try:
            LAST_RESULTS = run_bass_kernel_spmd(
                nc, in_maps, core_ids=list(range(8))
            )
            break
        except Exception as e:  # transient axon/PJRT hiccups
            last_exc = e
    else:
        raise last_exc
    res = LAST_RESULTS.results

    attn = np.concatenate([r["attn_o"] for r in res], axis=0)     # (B*A,T,T)
    output = np.concatenate([r["out_o"] for r in res], axis=0)    # (B,T,E)
    return output, attn


# revision 46
# speedup vs baseline: 1.0233x; 1.0233x over previous
"""Trainium2 Bass kernel for nn_MultiAtrAttention.

Math (reference):
  sf[b,a]  = atr[b,a] * X[b]
  score    = (sf @ sf^T)/sqrt(E), masked_fill(outer(mask)==0, 0)
  attn     = softmax(score - global_max, axis=-1)            -> output 2
  attn_out = attn @ sf                                        (B*A,T,E)
  output   = attn_out.reshape(B,T,A*E) @ W.T + bias           -> output 1

Identities used:
  score[b,a] = c1*Gm[b],  c1 = atr^2/sqrt(E),  Gm = Xm@Xm^T,
               Xm = diag(mask)@X  (exact masking via zeroed rows/cols)
  softmax is shift-invariant -> drop the global-max (scores <= ~25, exp
    stays finite in f32);  P = exp(c1*Gm) is SYMMETRIC
  D = rowsum(P) comes free from the ACT accumulator
  ao = attn_out[b,a] = diag(fac) @ P @ X[b],  fac = atr/D
  The reference reshape (B*A,T,E)->(B,T,A*E) regroups rows:
    output[b, 64a+r, :] = ao.reshape(64, A*E)[r] @ W.T + bias
  i.e. per (b,a): out rows 64a..64a+63 = ao[8r+j, e] contracted with
    W[:, j*E+e].  lhsT chunks are stride-8 column slices of ao^T, where
    ao^T = X^T @ P @ diag(fac)  (uses P symmetric; no transposes needed)

Precision: the Gram/softmax path is full f32 (PE fp32 matmul = 4
cycles/row).  The value path (U^T = X^T@P and the output projection) runs
in float32r (1 cycle/row, ~1.2e-4 input rounding), giving ~2.5e-4 max rel
error on both outputs -- the PE would otherwise be 2.5x over the ~62us/core
HBM roofline that bounds this kernel (attn output alone is 16.8 MB/core).

Sharding: batch-parallel, 2 batches per core x 8 cores.
"""

import sys

sys.path.insert(0, "/opt/trn_rl_repo")

import numpy as np

import concourse.mybir as mybir
import concourse.tile as tile
from concourse import bacc
from concourse.bass import ds, ts
from concourse.bass_utils import run_bass_kernel_spmd

B, T, E, A = 16, 512, 256, 8
NB = B // 8          # batches per core
TC = T // 128        # t-chunks of 128
EC = E // 128        # e-chunks of 128
NP = NB * A          # (b, a) pairs per core
RB = T // A          # 64 output rows per (b, a) block
F32 = mybir.dt.float32

LAST_RESULTS = None  # test.py reads exec_time/trace from here


def _build_program(ppb=8, apb=4, vpb=2, spb=4, fpb=4, opb=2, psgb=2, psub=3, pspb=2):
    nc = bacc.Bacc()

    xn = nc.dram_tensor("xn", (NB, T, E), mybir.dt.float32r, kind="ExternalInput")
    xmt = nc.dram_tensor("xmt", (NB, E, T), F32, kind="ExternalInput")
    wt = nc.dram_tensor("wt", (A * E, E), mybir.dt.float32r, kind="ExternalInput")
    bias = nc.dram_tensor("bias", (1, E), F32, kind="ExternalInput")
    coef = nc.dram_tensor("coef", (1, 2 * NP), F32, kind="ExternalInput")
    attn_o = nc.dram_tensor("attn_o", (NP, T, T), F32, kind="ExternalOutput")
    out_o = nc.dram_tensor("out_o", (NB, T, E), F32, kind="ExternalOutput")

    F32R = mybir.dt.float32r
    Exp = mybir.ActivationFunctionType.Exp
    Mult = mybir.AluOpType.mult

    with tile.TileContext(nc) as tc:
        with tc.tile_pool(name="const", bufs=1) as const:
            ones = const.tile([1, 128], F32)
            nc.vector.memset(ones[:], 1.0)
            coef_row = const.tile([1, 2 * NP], F32)
            nc.sync.dma_start(coef_row[:], coef[:])
            bias_row = const.tile([1, E], F32)
            nc.sync.dma_start(bias_row[:], bias[:])
            coefb = const.tile([128, 2 * NP], F32)
            biasb = const.tile([128, E], F32)

            # broadcast per-pair scalars + bias to all 128 partitions
            with tc.tile_pool(name="setup_ps", bufs=1, space="PSUM") as sps:
                t1 = sps.tile([128, 2 * NP], F32, tag="c")
                nc.tensor.matmul(t1[:], ones[:], coef_row[:])
                nc.vector.tensor_copy(coefb[:], t1[:])
                t2 = sps.tile([128, E], F32, tag="b")
                nc.tensor.matmul(t2[:], ones[:], bias_row[:])
                nc.vector.tensor_copy(biasb[:], t2[:])

            # persistent operands; xmt gates the Gram matmuls -> load first
            xmt_sb = const.tile([128, NB, EC, T], F32)
            xn_sb = const.tile([128, NB, TC, E], F32R)
            wt_sb = const.tile([128, A * EC, E], F32R)
            for b in range(NB):
                nc.sync.dma_start(
                    xmt_sb[:, b], xmt[b].rearrange("(ec p) t -> p ec t", p=128)
                )
            for b in range(NB):
                nc.sync.dma_start(
                    xn_sb[:, b], xn[b].rearrange("(tc p) e -> p tc e", p=128)
                )
            nc.sync.dma_start(wt_sb[:], wt.rearrange("(k p) o -> p k o", p=128))
            g_sb = const.tile([128, NB, TC, T], F32)

            with (
                tc.tile_pool(name="ppool", bufs=ppb) as ppool,
                tc.tile_pool(name="apool", bufs=apb) as apool,
                tc.tile_pool(name="vpool", bufs=vpb) as vpool,
                tc.tile_pool(name="spool", bufs=spb) as spool,
                tc.tile_pool(name="fpool", bufs=fpb) as fpool,
                tc.tile_pool(name="opool", bufs=opb) as opool,
                tc.tile_pool(name="psg", bufs=psgb, space="PSUM") as psg,
                tc.tile_pool(name="psu", bufs=psub, space="PSUM") as psu,
                tc.tile_pool(name="psp", bufs=pspb, space="PSUM") as psp,
            ):
                # ---- Gram matrices Gm[b] = Xm @ Xm^T (shared by all attrs)
                for b in range(NB):
                    for ti in range(TC):
                        pg = psg.tile([128, T], F32, tag="pg")
                        for ec in range(EC):
                            nc.tensor.matmul(
                                pg[:],
                                xmt_sb[:, b, ec, ts(ti, 128)],
                                xmt_sb[:, b, ec, :],
                                start=(ec == 0),
                                stop=(ec == EC - 1),
                            )
                        nc.vector.tensor_copy(g_sb[:, b, ti, :], pg[:])

                def emit_proj(v2, a):
                    # out[b, 64a+r, o] = sum_{j,e} ao_b[8r+j, e]*W[o, j*E+e]
                    po = psp.tile([128, E], F32, tag="po")
                    for kc in range(A * EC):
                        jj, ec = kc // EC, kc % EC
                        nc.tensor.matmul(
                            po[:],
                            v2[:, ec, :, :].rearrange("p b t -> p (b t)")[
                                :, jj : jj + 8 * (NB * RB - 1) + 1 : 8
                            ],
                            wt_sb[:, kc, :],
                            start=(kc == 0),
                            stop=(kc == A * EC - 1),
                        )
                    out_sb = opool.tile([128, E], F32, tag="out")
                    nc.vector.tensor_tensor(
                        out_sb[:], po[:], biasb[:], op=mybir.AluOpType.add
                    )
                    for bb in range(NB):
                        nc.sync.dma_start(
                            out_o[bb, ds(RB * a, RB), :],
                            out_sb[bb * RB : (bb + 1) * RB, :],
                        )

                prev_v2, prev_a = None, None
                for a in range(A):
                    v2 = vpool.tile([128, EC, NB, T], F32R, tag="v2")
                    for b in range(NB):
                        j = b * A + a
                        # ---- P = exp(c1*Gm); D = rowsum(P) fused on ACT
                        p_sb = ppool.tile([128, TC, T], F32R, tag="p")
                        d_sb = spool.tile([128, TC], F32, tag="d")
                        for ti in range(TC):
                            nc.scalar.activation(
                                p_sb[:, ti, :],
                                g_sb[:, b, ti, :],
                                Exp,
                                scale=coefb[:, j : j + 1],
                                accum_out=d_sb[:, ti : ti + 1],
                            )
                        rd_sb = spool.tile([128, TC], F32, tag="rd")
                        fac_sb = spool.tile([128, TC], F32, tag="fac")
                        facr = fpool.tile([1, T], F32, tag="facr")
                        fr = fpool.tile([128, T], F32, tag="fr")
                        # latency-critical chain: gates v' -> proj; boost its
                        # priority so it jumps same-engine throughput work
                        with tc.high_priority(offset=60):
                            nc.vector.reciprocal(rd_sb[:], d_sb[:])
                            nc.vector.tensor_scalar_mul(
                                fac_sb[:], rd_sb[:], coefb[:, NP + j : NP + j + 1]
                            )
                            # (128,TC) column -> (1,512) row in p-major order
                            nc.sync.dma_start(facr[:], fac_sb[:])
                            nc.gpsimd.partition_broadcast(fr[:], facr[:])
                        # natural-t view of the p-major row: t = c*128 + p
                        fr_nat = fr[:, :].rearrange("e (p c) -> e c p", p=128, c=TC)

                        # ---- ao^T = X^T @ P @ diag(fac)   (P symmetric)
                        for ec in range(EC):
                            pu = psu.tile([128, T], F32, tag="pu")
                            for k in range(TC):
                                nc.tensor.matmul(
                                    pu[:],
                                    xn_sb[:, b, k, ts(ec, 128)],
                                    p_sb[:, k, :],
                                    start=(k == 0),
                                    stop=(k == TC - 1),
                                )
                            with tc.high_priority(offset=60):
                                nc.vector.tensor_tensor(
                                    v2[:, ec, b, :].rearrange(
                                        "e (c p) -> e c p", c=TC, p=128
                                    ),
                                    pu[:, :].rearrange(
                                        "e (c p) -> e c p", c=TC, p=128
                                    ),
                                    fr_nat,
                                    op=Mult,
                                )

                        # ---- attn = P/D -> HBM (row scale; emitted after the
                        # PE-feeding chain so the scheduler favors the latter)
                        att_sb = apool.tile([128, TC, T], F32, tag="att")
                        with tc.high_priority(offset=55):
                            for ti in range(TC):
                                eng = nc.vector if ti < 2 else nc.gpsimd
                                eng.tensor_scalar_mul(
                                    att_sb[:, ti, :],
                                    p_sb[:, ti, :].bitcast(F32),
                                    rd_sb[:, ti : ti + 1],
                                )
                            for h in range(2):
                                nc.sync.dma_start(
                                    attn_o[j, ds(h * 256, 256)].rearrange(
                                        "(tc p) s -> p tc s", p=128
                                    ),
                                    att_sb[:, ds(h * 2, 2), :],
                                )

                    if prev_v2 is not None:
                        emit_proj(prev_v2, prev_a)
                    prev_v2, prev_a = v2, a
                emit_proj(prev_v2, prev_a)

    nc.finalize()
    return nc


_PROGRAM = None


def kernel(sent_feat, mask, atr_scores, W, b):
    global _PROGRAM, LAST_RESULTS
    sent_feat = np.asarray(sent_feat, dtype=np.float32)
    mask = np.asarray(mask)
    atr_scores = np.asarray(atr_scores, dtype=np.float32)
    W = np.asarray(W, dtype=np.float32)
    b = np.asarray(b, dtype=np.float32)

    xm = sent_feat * mask.astype(np.float32)[:, :, None]          # (B,T,E)
    xmt = np.ascontiguousarray(xm.transpose(0, 2, 1))             # (B,E,T)
    wt = np.ascontiguousarray(W.T)                                # (A*E, E)
    bias_row = np.ascontiguousarray(b.reshape(1, E))
    c1 = (atr_scores**2 / np.sqrt(E)).astype(np.float32)          # (B,A)
    c2 = atr_scores.astype(np.float32)

    if _PROGRAM is None:
        _PROGRAM = _build_program()
    nc = _PROGRAM

    in_maps = []
    for c in range(8):
        bs = slice(c * NB, (c + 1) * NB)
        coef = np.concatenate(
            [c1[bs].reshape(-1), c2[bs].reshape(-1)]
        ).reshape(1, 2 * NP)
        in_maps.append(
            {
                "xn": np.ascontiguousarray(sent_feat[bs]),
                "xmt": np.ascontiguousarray(xmt[bs]),
                "wt": wt,
                "bias": bias_row,
                "coef": np.ascontiguousarray(coef),
            }
        )

    last_exc = None
    for _attempt in range(3):
        try:
            LAST_RESULTS = run_bass_kernel_spmd(
                nc, in_maps, core_ids=list(range(8))
            )
            break
        except Exception as e:  # transient axon/PJRT hiccups
            last_exc = e
    else:
        raise last_exc
    res = LAST_RESULTS.results

    attn = np.concatenate([r["attn_o"] for r in res], axis=0)     # (B*A,T,T)
    output = np.concatenate([r["out_o"] for r in res], axis=0)    # (B,T,E)
    return output, attn
